# revision 2
# baseline (speedup 1.0000x reference)
"""Multi-head attention (B=2, S=2048, D=768, H=12) on 8 Trainium2 cores.

Sharding: core c -> batch b = c // 4, head-group g = c % 4 (3 heads of 12).
Each core computes Q/K/V projections for its head group, attention, and a
partial output (its head rows of Wo).  The host sums the 4 partials per
batch and adds bo.

Device kernel layout (per core):
  - x loaded from HBM with a casting DMA (fp32 -> bf16), PE-transposed to
    x^T so projections can contract over d_model.
  - Q^T, K^T produced per head as [64, 2048] tiles; V natural [2048, 192]
    with a ones column per head (softmax denominator rides the PV matmul).
  - scores computed transposed: S^T[k, q] = K Q^T, exp on the scalar engine
    (scale = 1/sqrt(64)), PV matmul V_aug^T @ P^T accumulates O^T[65, q]
    where row 64 is the softmax denominator.
  - normalize O^T with reciprocal + partition-broadcast, then the Wo
    row-shard matmul produces the partial output [2048, 768] in fp32.
"""

import sys

for _p in ("/opt/trn_rl_repo",):
    if _p not in sys.path:
        sys.path.append(_p)

import numpy as np

B = 2
S = 2048
D = 768
H = 12
DK = 64
HG = 3            # heads per core
HD = HG * DK      # 192
P = 128
NS = S // P       # 16 s-tiles
ND = D // P       # 6 d-chunks
NQ = S // 512     # 4 q-chunks of 512

_CACHE = {}


def _build_nc(use_bias_qkv):
    import concourse.bacc as bacc
    import concourse.tile as tile
    from concourse import mybir
    from concourse.masks import make_identity
    from contextlib import ExitStack

    BF = mybir.dt.bfloat16
    F32 = mybir.dt.float32

    nc = bacc.Bacc("TRN2", target_bir_lowering=False, debug=False)

    xq = nc.dram_tensor("xq", [S, D], F32, kind="ExternalInput").ap()
    xk = nc.dram_tensor("xk", [S, D], F32, kind="ExternalInput").ap()
    xv = nc.dram_tensor("xv", [S, D], F32, kind="ExternalInput").ap()
    wq = nc.dram_tensor("wq", [D, HD], F32, kind="ExternalInput").ap()
    wk = nc.dram_tensor("wk", [D, HD], F32, kind="ExternalInput").ap()
    wv = nc.dram_tensor("wv", [D, HD], F32, kind="ExternalInput").ap()
    wo = nc.dram_tensor("wo", [HD, D], F32, kind="ExternalInput").ap()
    bqkv = nc.dram_tensor("bqkv", [3, HD], F32, kind="ExternalInput").ap()
    y = nc.dram_tensor("y", [S, D], F32, kind="ExternalOutput").ap()

    with tile.TileContext(nc) as tc, ExitStack() as ctx:
        consts = ctx.enter_context(tc.tile_pool(name="consts", bufs=1))
        ident = consts.tile([P, P], BF)
        make_identity(nc, ident)

        # ---- weights: load fp32, cast to bf16 ----
        wpool = ctx.enter_context(tc.tile_pool(name="weights", bufs=1))
        w_bf = {}
        for name, w in (("wq", wq), ("wk", wk), ("wv", wv)):
            wf = wpool.tile([P, ND, HD], F32, tag=f"{name}_f32")
            nc.sync.dma_start(out=wf, in_=w.rearrange("(nd p) h -> p nd h", p=P))
            wb = wpool.tile([P, ND, HD], BF, tag=f"{name}_bf")
            nc.any.tensor_copy(out=wb, in_=wf)
            w_bf[name] = wb
        wo_f1 = wpool.tile([P, D], F32, tag="wo_f1")
        nc.sync.dma_start(out=wo_f1, in_=wo[0:P, :])
        wo_f2 = wpool.tile([DK, D], F32, tag="wo_f2")
        nc.sync.dma_start(out=wo_f2, in_=wo[P:HD, :])
        wo_b1 = wpool.tile([P, D], BF, tag="wo_b1")
        nc.any.tensor_copy(out=wo_b1, in_=wo_f1)
        wo_b2 = wpool.tile([DK, D], BF, tag="wo_b2")
        nc.any.tensor_copy(out=wo_b2, in_=wo_f2)

        bias_a = {}
        bias_b = {}
        if use_bias_qkv:
            # per-partition bias columns for the transposed projections
            for i, name in enumerate(("wq", "wk", "wv")):
                ba = wpool.tile([P, 1], F32, tag=f"ba_{name}")
                nc.sync.dma_start(out=ba, in_=bqkv[i, 0:P].rearrange("p -> p 1"))
                bb = wpool.tile([DK, 1], F32, tag=f"bb_{name}")
                nc.sync.dma_start(out=bb, in_=bqkv[i, P:HD].rearrange("p -> p 1"))
                bias_a[name] = ba
                bias_b[name] = bb

        # ---- persistent activation tiles ----
        apool = ctx.enter_context(tc.tile_pool(name="acts", bufs=1))
        QT = [apool.tile([DK, S], BF, tag=f"qt{h}", name=f"qt{h}") for h in range(HG)]
        KT = [apool.tile([DK, S], BF, tag=f"kt{h}", name=f"kt{h}") for h in range(HG)]
        V = apool.tile([P, NS, 3 * 65], BF, tag="v")
        nc.vector.memset(V[:, :, 64 : 3 * 65 : 65], 1.0)
        OC1 = apool.tile([P, S], BF, tag="oc1")    # heads 0,1 of O^T (normalized)
        OC2 = apool.tile([DK, S], BF, tag="oc2")   # head 2

        # ================= phase 1: transpose + projections =================
        with tc.tile_pool(name="stage", bufs=2) as stage_pool, \
             tc.tile_pool(name="xt", bufs=2) as xt_pool, \
             tc.tile_pool(name="tp_ps", bufs=2, space="PSUM") as tp_pool, \
             tc.tile_pool(name="mm_ps", bufs=2, space="PSUM") as mm_pool, \
             tc.tile_pool(name="mmb_ps", bufs=2, space="PSUM") as mmb_pool:

            def transpose_block(x_dram, sb, stage_tag):
                """load s-block sb (512 rows) of x and produce x^T chunks
                [128(d), ND, 512(s)] in bf16."""
                stg = stage_pool.tile([P, 4, D], BF, tag=stage_tag)
                nc.gpsimd.dma_start(
                    out=stg,
                    in_=x_dram.rearrange("(n p) m -> p n m", p=P)[
                        :, sb * 4 : (sb + 1) * 4, :
                    ],
                )
                xt = xt_pool.tile([P, ND, 512], BF, tag="xt")
                for d in range(ND):
                    tp = tp_pool.tile([P, 512], BF, tag="tp")
                    for j in range(4):
                        nc.tensor.transpose(
                            tp[:, j * P : (j + 1) * P],
                            stg[:, j, d * P : (d + 1) * P],
                            ident,
                        )
                    nc.any.tensor_copy(out=xt[:, d, :], in_=tp)
                return xt

            # Q^T and K^T:  [192, 512] per s-block = W^T @ x^T
            for name, x_dram, dstT in (("wq", xq, QT), ("wk", xk, KT)):
                wb = w_bf[name]
                for sb in range(NQ):
                    xt = transpose_block(x_dram, sb, f"stage")
                    psA = mm_pool.tile([P, 512], F32, tag="mm")
                    psB = mmb_pool.tile([DK, 512], F32, tag="mmb")
                    for d in range(ND):
                        nc.tensor.matmul(
                            psA, wb[:, d, 0:P], xt[:, d, :],
                            start=(d == 0), stop=(d == ND - 1),
                        )
                        nc.tensor.matmul(
                            psB, wb[:, d, P:HD], xt[:, d, :],
                            start=(d == 0), stop=(d == ND - 1),
                        )
                    sl = slice(sb * 512, (sb + 1) * 512)
                    if use_bias_qkv:
                        nc.vector.tensor_scalar_add(dstT[0][:, sl], psA[0:DK, :], bias_a[name][0:DK])
                        nc.vector.tensor_scalar_add(dstT[1][:, sl], psA[DK:P, :], bias_a[name][DK:P])
                        nc.vector.tensor_scalar_add(dstT[2][:, sl], psB, bias_b[name])
                    else:
                        nc.any.tensor_copy(out=dstT[0][:, sl], in_=psA[0:DK, :])
                        nc.any.tensor_copy(out=dstT[1][:, sl], in_=psA[DK:P, :])
                        nc.any.tensor_copy(out=dstT[2][:, sl], in_=psB)

            # V natural: [128(s), 192] per s-tile = x @ Wv
            wb = w_bf["wv"]
            for sb in range(NQ):
                xt = transpose_block(xv, sb, "stage")
                for j in range(4):
                    st = sb * 4 + j
                    psV = mm_pool.tile([P, HD], F32, tag="mm")
                    for d in range(ND):
                        nc.tensor.matmul(
                            psV, xt[:, d, j * P : (j + 1) * P], wb[:, d, :],
                            start=(d == 0), stop=(d == ND - 1),
                        )
                    for h in range(HG):
                        nc.any.tensor_copy(
                            out=V[:, st, h * 65 : h * 65 + 64],
                            in_=psV[:, h * DK : (h + 1) * DK],
                        )

        # ================= phase 2: attention per head =================
        with tc.tile_pool(name="s_ps", bufs=1, space="PSUM") as s_pool, \
             tc.tile_pool(name="ot_ps", bufs=1, space="PSUM") as ot_pool, \
             tc.tile_pool(name="pt", bufs=2) as pt_pool, \
             tc.tile_pool(name="nrm", bufs=2) as nrm_pool:
            from concourse import mybir as mb

            for h in range(HG):
                ot = ot_pool.tile([65, S], F32, tag="ot")
                for kt in range(NS):
                    s_ps = s_pool.tile([P, S], F32, tag="s")
                    for n in range(NQ):
                        nc.tensor.matmul(
                            s_ps[:, n * 512 : (n + 1) * 512],
                            KT[h][:, kt * P : (kt + 1) * P],
                            QT[h][:, n * 512 : (n + 1) * 512],
                            start=True, stop=True,
                        )
                    pt = pt_pool.tile([P, S], BF, tag="pt")
                    nc.scalar.activation(
                        pt, s_ps, mb.ActivationFunctionType.Exp,
                        bias=0.0, scale=0.125,
                    )
                    for n in range(NQ):
                        nc.tensor.matmul(
                            ot[:, n * 512 : (n + 1) * 512],
                            V[:, kt, h * 65 : (h + 1) * 65],
                            pt[:, n * 512 : (n + 1) * 512],
                            start=(kt == 0), stop=(kt == NS - 1),
                        )
                # normalize: O^T[j, q] * (1 / denom[q])
                recip = nrm_pool.tile([1, S], F32, tag="recip")
                nc.vector.reciprocal(recip, ot[64:65, :])
                rbc = nrm_pool.tile([DK, S], F32, tag="rbc")
                nc.gpsimd.partition_broadcast(rbc, recip)
                dst = OC1[0:DK, :] if h == 0 else (OC1[DK:P, :] if h == 1 else OC2)
                nc.vector.tensor_mul(dst, ot[0:DK, :], rbc)

        # ================= phase 3: partial Wo =================
        with tc.tile_pool(name="y_ps", bufs=2, space="PSUM") as y_pool, \
             tc.tile_pool(name="y_sb", bufs=2) as ysb_pool:
            y_r = y.rearrange("(n p) m -> n p m", p=P)
            for st in range(NS):
                y_ps = y_pool.tile([P, D], F32, tag="y")
                sl = slice(st * P, (st + 1) * P)
                for n0, nn in ((0, 512), (512, 256)):
                    nc.tensor.matmul(
                        y_ps[:, n0 : n0 + nn], OC1[:, sl], wo_b1[:, n0 : n0 + nn],
                        start=True, stop=False,
                    )
                    nc.tensor.matmul(
                        y_ps[:, n0 : n0 + nn], OC2[:, sl], wo_b2[:, n0 : n0 + nn],
                        start=False, stop=True,
                    )
                y_sb = ysb_pool.tile([P, D], F32, tag="ysb")
                nc.any.tensor_copy(out=y_sb, in_=y_ps)
                nc.sync.dma_start(out=y_r[st], in_=y_sb)

    nc.compile()
    return nc


def kernel(query, key, value, Wq, bq, Wk, bk, Wv, bv, Wo, bo, **_ignored):
    from concourse.bass_utils import run_bass_kernel_spmd

    query = np.asarray(query, dtype=np.float32)
    key = np.asarray(key, dtype=np.float32)
    value = np.asarray(value, dtype=np.float32)
    Wq = np.asarray(Wq, dtype=np.float32)
    Wk = np.asarray(Wk, dtype=np.float32)
    Wv = np.asarray(Wv, dtype=np.float32)
    Wo = np.asarray(Wo, dtype=np.float32)
    bq = np.asarray(bq, dtype=np.float32)
    bk = np.asarray(bk, dtype=np.float32)
    bv = np.asarray(bv, dtype=np.float32)
    bo = np.asarray(bo, dtype=np.float32)

    use_bias_qkv = bool(np.any(bq) or np.any(bk) or np.any(bv))
    if "nc" not in _CACHE or _CACHE.get("bias") != use_bias_qkv:
        _CACHE["nc"] = _build_nc(use_bias_qkv)
        _CACHE["bias"] = use_bias_qkv
    nc = _CACHE["nc"]

    in_maps = []
    for c in range(8):
        b, g = divmod(c, 4)
        hs = slice(g * HD, (g + 1) * HD)
        in_maps.append({
            "xq": np.ascontiguousarray(query[b]),
            "xk": np.ascontiguousarray(key[b]),
            "xv": np.ascontiguousarray(value[b]),
            "wq": np.ascontiguousarray(Wq[:, hs]),
            "wk": np.ascontiguousarray(Wk[:, hs]),
            "wv": np.ascontiguousarray(Wv[:, hs]),
            "wo": np.ascontiguousarray(Wo[hs, :]),
            "bqkv": np.ascontiguousarray(
                np.stack([bq[hs], bk[hs], bv[hs]]).astype(np.float32)
            ),
        })

    res = run_bass_kernel_spmd(nc, in_maps, core_ids=list(range(8)), **_CACHE.get("run_kwargs", {}))
    _CACHE["last_result"] = res

    out = np.empty((B, S, D), dtype=np.float32)
    for b in range(B):
        acc = res.results[4 * b]["y"].astype(np.float32).copy()
        for g in range(1, 4):
            acc += res.results[4 * b + g]["y"]
        out[b] = acc + bo[None, :]
    return out


# revision 4
# speedup vs baseline: 1.4238x; 1.4238x over previous
"""Multi-head attention (B=2, S=2048, D=768, H=12) on 8 Trainium2 cores.

Sharding: core c -> batch b = c // 4, head-group g = c % 4 (3 heads of 12).
Each core computes Q/K/V projections for its head group, attention, and a
partial output (its head rows of Wo).  The host sums the 4 partials per
batch and adds bo.

Device kernel layout (per core):
  - x loaded from HBM with a casting DMA (fp32 -> bf16), PE-transposed to
    x^T so projections can contract over d_model.
  - Q^T, K^T produced per head as [64, 2048] tiles; V natural [2048, 192]
    with a ones column per head (softmax denominator rides the PV matmul).
  - scores computed transposed: S^T[k, q] = K Q^T, exp on the scalar engine
    (scale = 1/sqrt(64)), PV matmul V_aug^T @ P^T accumulates O^T[65, q]
    where row 64 is the softmax denominator.
  - normalize O^T with reciprocal + partition-broadcast, then the Wo
    row-shard matmul produces the partial output [2048, 768] in fp32.
"""

import sys

for _p in ("/opt/trn_rl_repo",):
    if _p not in sys.path:
        sys.path.append(_p)

import numpy as np

B = 2
S = 2048
D = 768
H = 12
DK = 64
HG = 3            # heads per core
HD = HG * DK      # 192
P = 128
NS = S // P       # 16 s-tiles
ND = D // P       # 6 d-chunks
NQ = S // 512     # 4 q-chunks of 512

_CACHE = {}


def _build_nc(use_bias_qkv):
    import concourse.bacc as bacc
    import concourse.tile as tile
    from concourse import mybir
    from concourse.masks import make_identity
    from contextlib import ExitStack

    BF = mybir.dt.bfloat16
    F32 = mybir.dt.float32

    nc = bacc.Bacc("TRN2", target_bir_lowering=False, debug=False)

    xq = nc.dram_tensor("xq", [S, D], F32, kind="ExternalInput").ap()
    xk = nc.dram_tensor("xk", [S, D], F32, kind="ExternalInput").ap()
    xv = nc.dram_tensor("xv", [S, D], F32, kind="ExternalInput").ap()
    wq = nc.dram_tensor("wq", [D, HD], F32, kind="ExternalInput").ap()
    wk = nc.dram_tensor("wk", [D, HD], F32, kind="ExternalInput").ap()
    wv = nc.dram_tensor("wv", [D, HD], F32, kind="ExternalInput").ap()
    wo = nc.dram_tensor("wo", [HD, D], F32, kind="ExternalInput").ap()
    bqkv = nc.dram_tensor("bqkv", [3, HD], F32, kind="ExternalInput").ap()
    y = nc.dram_tensor("y", [S, D], F32, kind="ExternalOutput").ap()

    with tile.TileContext(nc) as tc, ExitStack() as ctx:
        consts = ctx.enter_context(tc.tile_pool(name="consts", bufs=1))
        ident = consts.tile([P, P], BF)
        make_identity(nc, ident)

        # ---- weights: load fp32, cast to bf16 ----
        wpool = ctx.enter_context(tc.tile_pool(name="weights", bufs=1))
        w_bf = {}
        for name, w in (("wq", wq), ("wk", wk), ("wv", wv)):
            wf = wpool.tile([P, ND, HD], F32, tag=f"{name}_f32")
            nc.sync.dma_start(out=wf, in_=w.rearrange("(nd p) h -> p nd h", p=P))
            wb = wpool.tile([P, ND, HD], BF, tag=f"{name}_bf")
            nc.any.tensor_copy(out=wb, in_=wf)
            w_bf[name] = wb
        wo_f1 = wpool.tile([P, D], F32, tag="wo_f1")
        nc.sync.dma_start(out=wo_f1, in_=wo[0:P, :])
        wo_f2 = wpool.tile([DK, D], F32, tag="wo_f2")
        nc.sync.dma_start(out=wo_f2, in_=wo[P:HD, :])
        wo_b1 = wpool.tile([P, D], BF, tag="wo_b1")
        nc.any.tensor_copy(out=wo_b1, in_=wo_f1)
        wo_b2 = wpool.tile([DK, D], BF, tag="wo_b2")
        nc.any.tensor_copy(out=wo_b2, in_=wo_f2)

        bias_a = {}
        bias_b = {}
        if use_bias_qkv:
            # per-partition bias columns for the transposed projections
            for i, name in enumerate(("wq", "wk", "wv")):
                ba = wpool.tile([P, 1], F32, tag=f"ba_{name}")
                nc.sync.dma_start(out=ba, in_=bqkv[i, 0:P].rearrange("p -> p 1"))
                bb = wpool.tile([DK, 1], F32, tag=f"bb_{name}")
                nc.sync.dma_start(out=bb, in_=bqkv[i, P:HD].rearrange("p -> p 1"))
                bias_a[name] = ba
                bias_b[name] = bb

        # ---- persistent activation tiles ----
        apool = ctx.enter_context(tc.tile_pool(name="acts", bufs=1))
        QT = [apool.tile([DK, S], BF, tag=f"qt{h}", name=f"qt{h}") for h in range(HG)]
        KT = [apool.tile([DK, S], BF, tag=f"kt{h}", name=f"kt{h}") for h in range(HG)]
        V = apool.tile([P, NS, 3 * 65], BF, tag="v")
        nc.vector.memset(V[:, :, 64 : 3 * 65 : 65], 1.0)
        OC1 = apool.tile([P, S], BF, tag="oc1")    # heads 0,1 of O^T (normalized)
        OC2 = apool.tile([DK, S], BF, tag="oc2")   # head 2

        # ================= phase 1: transpose + projections =================
        with tc.tile_pool(name="stage", bufs=2) as stage_pool, \
             tc.tile_pool(name="xt", bufs=2) as xt_pool, \
             tc.tile_pool(name="tp_ps", bufs=2, space="PSUM") as tp_pool, \
             tc.tile_pool(name="mm_ps", bufs=2, space="PSUM") as mm_pool, \
             tc.tile_pool(name="mmb_ps", bufs=2, space="PSUM") as mmb_pool:

            def transpose_block(x_dram, sb, stage_tag):
                """load s-block sb (512 rows) of x and produce x^T chunks
                [128(d), ND, 512(s)] in bf16."""
                stg = stage_pool.tile([P, 4, D], BF, tag=stage_tag)
                nc.gpsimd.dma_start(
                    out=stg,
                    in_=x_dram.rearrange("(n p) m -> p n m", p=P)[
                        :, sb * 4 : (sb + 1) * 4, :
                    ],
                )
                xt = xt_pool.tile([P, ND, 512], BF, tag="xt")
                for d in range(ND):
                    tp = tp_pool.tile([P, 512], BF, tag="tp")
                    for j in range(4):
                        nc.tensor.transpose(
                            tp[:, j * P : (j + 1) * P],
                            stg[:, j, d * P : (d + 1) * P],
                            ident,
                        )
                    nc.any.tensor_copy(out=xt[:, d, :], in_=tp)
                return xt

            # Q^T and K^T:  [192, 512] per s-block = W^T @ x^T
            for name, x_dram, dstT in (("wq", xq, QT), ("wk", xk, KT)):
                wb = w_bf[name]
                for sb in range(NQ):
                    xt = transpose_block(x_dram, sb, f"stage")
                    psA = mm_pool.tile([P, 512], F32, tag="mm")
                    psB = mmb_pool.tile([DK, 512], F32, tag="mmb")
                    for d in range(ND):
                        nc.tensor.matmul(
                            psA, wb[:, d, 0:P], xt[:, d, :],
                            start=(d == 0), stop=(d == ND - 1),
                        )
                        nc.tensor.matmul(
                            psB, wb[:, d, P:HD], xt[:, d, :],
                            start=(d == 0), stop=(d == ND - 1),
                        )
                    sl = slice(sb * 512, (sb + 1) * 512)
                    if use_bias_qkv:
                        nc.vector.tensor_scalar_add(dstT[0][:, sl], psA[0:DK, :], bias_a[name][0:DK])
                        nc.vector.tensor_scalar_add(dstT[1][:, sl], psA[DK:P, :], bias_a[name][DK:P])
                        nc.vector.tensor_scalar_add(dstT[2][:, sl], psB, bias_b[name])
                    else:
                        nc.any.tensor_copy(out=dstT[0][:, sl], in_=psA[0:DK, :])
                        nc.any.tensor_copy(out=dstT[1][:, sl], in_=psA[DK:P, :])
                        nc.any.tensor_copy(out=dstT[2][:, sl], in_=psB)

            # V natural: [128(s), 192] per s-tile = x @ Wv
            wb = w_bf["wv"]
            for sb in range(NQ):
                xt = transpose_block(xv, sb, "stage")
                for j in range(4):
                    st = sb * 4 + j
                    psV = mm_pool.tile([P, HD], F32, tag="mm")
                    for d in range(ND):
                        nc.tensor.matmul(
                            psV, xt[:, d, j * P : (j + 1) * P], wb[:, d, :],
                            start=(d == 0), stop=(d == ND - 1),
                        )
                    for h in range(HG):
                        nc.any.tensor_copy(
                            out=V[:, st, h * 65 : h * 65 + 64],
                            in_=psV[:, h * DK : (h + 1) * DK],
                        )

        # ================= phase 2: attention per head =================
        # q is processed in halves of 1024 so the scores PSUM tile can be
        # double-buffered (2 banks x 2 bufs + 4 banks for O^T = 8 banks):
        # the PE then streams scores/PV matmuls back-to-back instead of
        # stalling ~2us per k-tile on the exp, which kept HAM at K=4/8.
        QH = 1024
        with tc.tile_pool(name="s_ps", bufs=2, space="PSUM") as s_pool, \
             tc.tile_pool(name="ot_ps", bufs=1, space="PSUM") as ot_pool, \
             tc.tile_pool(name="pt", bufs=3) as pt_pool, \
             tc.tile_pool(name="nrm", bufs=2) as nrm_pool:
            from concourse import mybir as mb

            for h in range(HG):
                ot = ot_pool.tile([65, S], F32, tag="ot")
                for kt in range(NS):
                    for qh in range(S // QH):
                        s_ps = s_pool.tile([P, QH], F32, tag="s")
                        for n in range(QH // 512):
                            q0 = qh * QH + n * 512
                            nc.tensor.matmul(
                                s_ps[:, n * 512 : (n + 1) * 512],
                                KT[h][:, kt * P : (kt + 1) * P],
                                QT[h][:, q0 : q0 + 512],
                                start=True, stop=True,
                            )
                        pt = pt_pool.tile([P, QH], BF, tag="pt")
                        nc.scalar.activation(
                            pt, s_ps, mb.ActivationFunctionType.Exp,
                            bias=0.0, scale=0.125,
                        )
                        for n in range(QH // 512):
                            q0 = qh * QH + n * 512
                            nc.tensor.matmul(
                                ot[:, q0 : q0 + 512],
                                V[:, kt, h * 65 : (h + 1) * 65],
                                pt[:, n * 512 : (n + 1) * 512],
                                start=(kt == 0), stop=(kt == NS - 1),
                            )
                # Evict O^T_aug to SBUF (frees the PSUM tile for the next
                # head) and normalize off the critical path.
                ot_sb = nrm_pool.tile([65, S], F32, tag="ot_sb")
                nc.vector.tensor_copy(out=ot_sb, in_=ot)
                recip = nrm_pool.tile([1, S], F32, tag="recip")
                nc.vector.reciprocal(recip, ot_sb[64:65, :])
                rbc = nrm_pool.tile([DK, S], F32, tag="rbc")
                nc.gpsimd.partition_broadcast(rbc, recip)
                dst = OC1[0:DK, :] if h == 0 else (OC1[DK:P, :] if h == 1 else OC2)
                nc.vector.tensor_mul(dst, ot_sb[0:DK, :], rbc)

        # ================= phase 3: partial Wo =================
        with tc.tile_pool(name="y_ps", bufs=2, space="PSUM") as y_pool, \
             tc.tile_pool(name="y_sb", bufs=2) as ysb_pool:
            y_r = y.rearrange("(n p) m -> n p m", p=P)
            for st in range(NS):
                y_ps = y_pool.tile([P, D], F32, tag="y")
                sl = slice(st * P, (st + 1) * P)
                for n0, nn in ((0, 512), (512, 256)):
                    nc.tensor.matmul(
                        y_ps[:, n0 : n0 + nn], OC1[:, sl], wo_b1[:, n0 : n0 + nn],
                        start=True, stop=False,
                    )
                    nc.tensor.matmul(
                        y_ps[:, n0 : n0 + nn], OC2[:, sl], wo_b2[:, n0 : n0 + nn],
                        start=False, stop=True,
                    )
                y_sb = ysb_pool.tile([P, D], F32, tag="ysb")
                nc.any.tensor_copy(out=y_sb, in_=y_ps)
                nc.sync.dma_start(out=y_r[st], in_=y_sb)

    nc.compile()
    return nc


def kernel(query, key, value, Wq, bq, Wk, bk, Wv, bv, Wo, bo, **_ignored):
    from concourse.bass_utils import run_bass_kernel_spmd

    query = np.asarray(query, dtype=np.float32)
    key = np.asarray(key, dtype=np.float32)
    value = np.asarray(value, dtype=np.float32)
    Wq = np.asarray(Wq, dtype=np.float32)
    Wk = np.asarray(Wk, dtype=np.float32)
    Wv = np.asarray(Wv, dtype=np.float32)
    Wo = np.asarray(Wo, dtype=np.float32)
    bq = np.asarray(bq, dtype=np.float32)
    bk = np.asarray(bk, dtype=np.float32)
    bv = np.asarray(bv, dtype=np.float32)
    bo = np.asarray(bo, dtype=np.float32)

    use_bias_qkv = bool(np.any(bq) or np.any(bk) or np.any(bv))
    if "nc" not in _CACHE or _CACHE.get("bias") != use_bias_qkv:
        _CACHE["nc"] = _build_nc(use_bias_qkv)
        _CACHE["bias"] = use_bias_qkv
    nc = _CACHE["nc"]

    in_maps = []
    for c in range(8):
        b, g = divmod(c, 4)
        hs = slice(g * HD, (g + 1) * HD)
        in_maps.append({
            "xq": np.ascontiguousarray(query[b]),
            "xk": np.ascontiguousarray(key[b]),
            "xv": np.ascontiguousarray(value[b]),
            "wq": np.ascontiguousarray(Wq[:, hs]),
            "wk": np.ascontiguousarray(Wk[:, hs]),
            "wv": np.ascontiguousarray(Wv[:, hs]),
            "wo": np.ascontiguousarray(Wo[hs, :]),
            "bqkv": np.ascontiguousarray(
                np.stack([bq[hs], bk[hs], bv[hs]]).astype(np.float32)
            ),
        })

    res = run_bass_kernel_spmd(nc, in_maps, core_ids=list(range(8)), **_CACHE.get("run_kwargs", {}))
    _CACHE["last_result"] = res

    out = np.empty((B, S, D), dtype=np.float32)
    for b in range(B):
        acc = res.results[4 * b]["y"].astype(np.float32).copy()
        for g in range(1, 4):
            acc += res.results[4 * b + g]["y"]
        out[b] = acc + bo[None, :]
    return out


# revision 5
# speedup vs baseline: 1.5821x; 1.1112x over previous
"""Multi-head attention (B=2, S=2048, D=768, H=12) on 8 Trainium2 cores.

Sharding: core c -> batch b = c // 4, head-group g = c % 4 (3 heads of 12).
Each core computes Q/K/V projections for its head group, attention, and a
partial output (its head rows of Wo).  The host sums the 4 partials per
batch and adds bo.

Device kernel layout (per core):
  - x loaded from HBM with a casting DMA (fp32 -> bf16) into a permuted
    s-layout (partition p holds rows 4p..4p+3 of each 512-row block, so DMA
    descriptors are 12KB contiguous reads).  Attention is invariant to a
    consistent permutation of q and k; the output DMA unpermutes.
  - x^T via PE transposes; Q^T, K^T per head as [64, 2048] bf16 tiles; V
    natural [2048, 3*65] with a ones column per head (softmax denominator
    rides the PV matmul).
  - scores computed transposed: S^T[k, q] = K Q^T, exp on the scalar engine
    (scale = 1/sqrt(64)), PV matmul V_aug^T @ P^T accumulates O^T[65, q]
    where row 64 is the softmax denominator.  q is processed in halves of
    1024 and heads 0/1 are interleaved in the k loop so the PE streams
    back-to-back (keeps HAM at K=8/8) while ACT does the exps.
  - normalize O^T with reciprocal + partition-broadcast off the critical
    path, then the Wo row-shard matmul (interleaved with the solo head's
    attention where possible) produces the partial [2048, 768] fp32 output.
"""

import sys

for _p in ("/opt/trn_rl_repo",):
    if _p not in sys.path:
        sys.path.append(_p)

import numpy as np

B = 2
S = 2048
D = 768
H = 12
DK = 64
HG = 3            # heads per core
HD = HG * DK      # 192
P = 128
NS = S // P       # 16 s-tiles
ND = D // P       # 6 d-chunks
NB = S // 512     # 4 s-blocks
QH = 1024         # q half

_CACHE = {}


def _build_nc(use_bias_qkv):
    import concourse.bacc as bacc
    import concourse.tile as tile
    from concourse import mybir
    from concourse.masks import make_identity
    from contextlib import ExitStack

    BF = mybir.dt.bfloat16
    F32 = mybir.dt.float32
    EXP = mybir.ActivationFunctionType.Exp

    nc = bacc.Bacc("TRN2", target_bir_lowering=False, debug=False)

    xq = nc.dram_tensor("xq", [S, D], F32, kind="ExternalInput").ap()
    xk = nc.dram_tensor("xk", [S, D], F32, kind="ExternalInput").ap()
    xv = nc.dram_tensor("xv", [S, D], F32, kind="ExternalInput").ap()
    wq = nc.dram_tensor("wq", [D, HD], F32, kind="ExternalInput").ap()
    wk = nc.dram_tensor("wk", [D, HD], F32, kind="ExternalInput").ap()
    wv = nc.dram_tensor("wv", [D, HD], F32, kind="ExternalInput").ap()
    wo = nc.dram_tensor("wo", [HD, D], F32, kind="ExternalInput").ap()
    bqkv = nc.dram_tensor("bqkv", [3, HD], F32, kind="ExternalInput").ap()
    y = nc.dram_tensor("y", [S, D], F32, kind="ExternalOutput").ap()

    with tile.TileContext(nc) as tc, ExitStack() as ctx:
        consts = ctx.enter_context(tc.tile_pool(name="consts", bufs=1))
        wpool = ctx.enter_context(tc.tile_pool(name="weights", bufs=1))
        apool = ctx.enter_context(tc.tile_pool(name="acts", bufs=1))

        ident = consts.tile([P, P], BF)

        # persistent activation tiles
        QT = [apool.tile([DK, S], BF, tag=f"qt{h}", name=f"qt{h}") for h in range(HG)]
        KT = [apool.tile([DK, S], BF, tag=f"kt{h}", name=f"kt{h}") for h in range(HG)]
        V = apool.tile([P, NS, 3 * 65], BF, tag="v")
        OC1 = apool.tile([P, S], BF, tag="oc1")    # heads 0,1 of O^T (normalized)
        OC2 = apool.tile([DK, S], BF, tag="oc2")   # head 2

        # ================= phase 1: transpose + projections =================
        with tc.tile_pool(name="stage", bufs=2) as stage_pool, \
             tc.tile_pool(name="xt", bufs=2) as xt_pool, \
             tc.tile_pool(name="tp_ps", bufs=2, space="PSUM") as tp_pool, \
             tc.tile_pool(name="mm_ps", bufs=2, space="PSUM") as mm_pool, \
             tc.tile_pool(name="mmb_ps", bufs=2, space="PSUM") as mmb_pool:

            first = [True]

            def transpose_block(x_dram, sb):
                """Load s-block sb (512 rows) of x (permuted: partition p
                holds rows 4p..4p+3) and produce x^T chunks [128(d), ND,
                512(s')] bf16 where s' column n*128+p corresponds to row
                sb*512 + 4p + n."""
                stg = stage_pool.tile([P, 4, D], BF, tag="stage")
                nc.gpsimd.dma_start(
                    out=stg,
                    in_=x_dram.rearrange("(a p n) m -> a p n m", p=P, n=4)[sb],
                )
                if first[0]:
                    # emit identity + weight loads after the first DMA so
                    # nothing delays the pipeline head
                    first[0] = False
                    make_identity(nc, ident)
                    for name, w in (("wq", wq), ("wk", wk), ("wv", wv)):
                        wf = wpool.tile([P, ND, HD], F32, tag=f"{name}_f32", name=f"{name}_f32")
                        nc.sync.dma_start(out=wf, in_=w.rearrange("(nd p) h -> p nd h", p=P))
                        wb = wpool.tile([P, ND, HD], BF, tag=f"{name}_bf", name=f"{name}_bf")
                        nc.any.tensor_copy(out=wb, in_=wf)
                        w_bf[name] = wb
                    wo_f1 = wpool.tile([P, D], F32, tag="wo_f1")
                    nc.sync.dma_start(out=wo_f1, in_=wo[0:P, :])
                    wo_f2 = wpool.tile([DK, D], F32, tag="wo_f2")
                    nc.sync.dma_start(out=wo_f2, in_=wo[P:HD, :])
                    nc.any.tensor_copy(out=wo_b1, in_=wo_f1)
                    nc.any.tensor_copy(out=wo_b2, in_=wo_f2)
                    nc.vector.memset(V[:, :, 64 : 3 * 65 : 65], 1.0)
                    if use_bias_qkv:
                        for i, name in enumerate(("wq", "wk", "wv")):
                            ba = wpool.tile([P, 1], F32, tag=f"ba_{name}", name=f"ba_{name}")
                            nc.sync.dma_start(out=ba, in_=bqkv[i, 0:P].rearrange("p -> p 1"))
                            bb = wpool.tile([DK, 1], F32, tag=f"bb_{name}", name=f"bb_{name}")
                            nc.sync.dma_start(out=bb, in_=bqkv[i, P:HD].rearrange("p -> p 1"))
                            bias_a[name] = ba
                            bias_b[name] = bb
                xt = xt_pool.tile([P, ND, 512], BF, tag="xt")
                for d in range(ND):
                    tp = tp_pool.tile([P, 512], BF, tag="tp")
                    for j in range(4):
                        nc.tensor.transpose(
                            tp[:, j * P : (j + 1) * P],
                            stg[:, j, d * P : (d + 1) * P],
                            ident,
                        )
                    nc.any.tensor_copy(out=xt[:, d, :], in_=tp)
                return xt

            w_bf = {}
            bias_a = {}
            bias_b = {}
            wo_b1 = wpool.tile([P, D], BF, tag="wo_b1")
            wo_b2 = wpool.tile([DK, D], BF, tag="wo_b2")

            # K^T and Q^T:  [192, 512] per s-block = W^T @ x^T
            for name, x_dram, dstT in (("wk", xk, KT), ("wq", xq, QT)):
                for sb in range(NB):
                    xt = transpose_block(x_dram, sb)
                    wb = w_bf[name]
                    psA = mm_pool.tile([P, 512], F32, tag="mm")
                    psB = mmb_pool.tile([DK, 512], F32, tag="mmb")
                    for d in range(ND):
                        nc.tensor.matmul(
                            psA, wb[:, d, 0:P], xt[:, d, :],
                            start=(d == 0), stop=(d == ND - 1),
                        )
                        nc.tensor.matmul(
                            psB, wb[:, d, P:HD], xt[:, d, :],
                            start=(d == 0), stop=(d == ND - 1),
                        )
                    sl = slice(sb * 512, (sb + 1) * 512)
                    if use_bias_qkv:
                        nc.vector.tensor_scalar_add(dstT[0][:, sl], psA[0:DK, :], bias_a[name][0:DK])
                        nc.vector.tensor_scalar_add(dstT[1][:, sl], psA[DK:P, :], bias_a[name][DK:P])
                        nc.vector.tensor_scalar_add(dstT[2][:, sl], psB, bias_b[name])
                    else:
                        nc.any.tensor_copy(out=dstT[0][:, sl], in_=psA[0:DK, :])
                        nc.any.tensor_copy(out=dstT[1][:, sl], in_=psA[DK:P, :])
                        nc.any.tensor_copy(out=dstT[2][:, sl], in_=psB)

            # V natural: [128(s'), 192] per s-tile = x @ Wv
            for sb in range(NB):
                xt = transpose_block(xv, sb)
                wb = w_bf["wv"]
                for j in range(4):
                    st = sb * 4 + j
                    psV = mm_pool.tile([P, HD], F32, tag="mm")
                    for d in range(ND):
                        nc.tensor.matmul(
                            psV, xt[:, d, j * P : (j + 1) * P], wb[:, d, :],
                            start=(d == 0), stop=(d == ND - 1),
                        )
                    for h in range(HG):
                        nc.any.tensor_copy(
                            out=V[:, st, h * 65 : h * 65 + 64],
                            in_=psV[:, h * DK : (h + 1) * DK],
                        )

        # ============ phase 2+3: attention (+ interleaved Wo) ============
        with tc.tile_pool(name="s_ps", bufs=2, space="PSUM") as s_pool, \
             tc.tile_pool(name="ot_ps", bufs=2, space="PSUM") as ot_pool, \
             tc.tile_pool(name="pt", bufs=3) as pt_pool, \
             tc.tile_pool(name="nrm", bufs=2) as nrm_pool, \
             tc.tile_pool(name="y_sb", bufs=2) as ysb_pool:

            def scores_exp_pv(h, kt, qh, ot):
                s_ps = s_pool.tile([P, QH], F32, tag="s", name="s_ps")
                for n in range(QH // 512):
                    q0 = qh * QH + n * 512
                    nc.tensor.matmul(
                        s_ps[:, n * 512 : (n + 1) * 512],
                        KT[h][:, kt * P : (kt + 1) * P],
                        QT[h][:, q0 : q0 + 512],
                        start=True, stop=True,
                    )
                pt = pt_pool.tile([P, QH], BF, tag="pt", name="pt")
                nc.scalar.activation(pt, s_ps, EXP, bias=0.0, scale=0.125)
                for n in range(QH // 512):
                    nc.tensor.matmul(
                        ot[:, n * 512 : (n + 1) * 512],
                        V[:, kt, h * 65 : (h + 1) * 65],
                        pt[:, n * 512 : (n + 1) * 512],
                        start=(kt == 0), stop=(kt == NS - 1),
                    )

            def normalize(h, qh, ot):
                """O^T[j, q] /= denom[q]; writes the normalized bf16 head
                slice into OC1/OC2.  All off the ACT critical path."""
                osb = nrm_pool.tile([DK, QH], F32, tag="osb", name="osb")
                nc.vector.tensor_copy(out=osb, in_=ot[0:DK, :])
                den = nrm_pool.tile([1, QH], F32, tag="den", name="den")
                nc.vector.tensor_copy(out=den, in_=ot[64:65, :])
                recip = nrm_pool.tile([1, QH], F32, tag="recip", name="recip")
                nc.vector.reciprocal_approx_fast(recip, den)
                rbc = nrm_pool.tile([DK, QH], F32, tag="rbc", name="rbc")
                nc.gpsimd.partition_broadcast(rbc, recip)
                sl = slice(qh * QH, (qh + 1) * QH)
                dst = OC1[0:DK, sl] if h == 0 else (OC1[DK:P, sl] if h == 1 else OC2[:, sl])
                nc.vector.tensor_mul(dst, osb, rbc)

            y_r = y.rearrange("(a p n) m -> a n p m", p=P, n=4)

            def wo_tile(st):
                """Partial Wo for s-tile st (q columns st*128..+127, which are
                rows (st//4)*512 + 4p + (st%4))."""
                y_ps = ot_pool.tile([P, D], F32, tag="ot", name="y_ps")
                sl = slice(st * P, (st + 1) * P)
                for n0, nn in ((0, 512), (512, 256)):
                    nc.tensor.matmul(
                        y_ps[:, n0 : n0 + nn], OC1[:, sl], wo_b1[:, n0 : n0 + nn],
                        start=True, stop=False,
                    )
                    nc.tensor.matmul(
                        y_ps[:, n0 : n0 + nn], OC2[:, sl], wo_b2[:, n0 : n0 + nn],
                        start=False, stop=True,
                    )
                y_sb = ysb_pool.tile([P, D], F32, tag="ysb", name="y_sb")
                nc.vector.tensor_copy(out=y_sb, in_=y_ps)
                nc.sync.dma_start(out=y_r[st // 4, st % 4], in_=y_sb)

            for qh in range(S // QH):
                # paired heads 0,1: interleaved so PE streams while ACT exps
                ot01 = [
                    ot_pool.tile([65, QH], F32, tag="ot", name=f"ot{h}_{qh}")
                    for h in range(2)
                ]
                for kt in range(NS):
                    for h in range(2):
                        scores_exp_pv(h, kt, qh, ot01[h])
                for h in range(2):
                    normalize(h, qh, ot01[h])
                # solo head 2, with the previous q-half's Wo interleaved
                ot2 = ot_pool.tile([65, QH], F32, tag="ot", name=f"ot2_{qh}")
                for kt in range(NS):
                    scores_exp_pv(2, kt, qh, ot2)
                    if qh > 0 and kt % 2 == 1:
                        wo_tile((qh - 1) * (NS // 2) + kt // 2)
                normalize(2, qh, ot2)
            # final q-half's Wo
            for st in range(NS // 2, NS):
                wo_tile(st)

    nc.compile()
    return nc


def kernel(query, key, value, Wq, bq, Wk, bk, Wv, bv, Wo, bo, **_ignored):
    from concourse.bass_utils import run_bass_kernel_spmd

    query = np.asarray(query, dtype=np.float32)
    key = np.asarray(key, dtype=np.float32)
    value = np.asarray(value, dtype=np.float32)
    Wq = np.asarray(Wq, dtype=np.float32)
    Wk = np.asarray(Wk, dtype=np.float32)
    Wv = np.asarray(Wv, dtype=np.float32)
    Wo = np.asarray(Wo, dtype=np.float32)
    bq = np.asarray(bq, dtype=np.float32)
    bk = np.asarray(bk, dtype=np.float32)
    bv = np.asarray(bv, dtype=np.float32)
    bo = np.asarray(bo, dtype=np.float32)

    use_bias_qkv = bool(np.any(bq) or np.any(bk) or np.any(bv))
    if "nc" not in _CACHE or _CACHE.get("bias") != use_bias_qkv:
        _CACHE["nc"] = _build_nc(use_bias_qkv)
        _CACHE["bias"] = use_bias_qkv
    nc = _CACHE["nc"]

    in_maps = []
    for c in range(8):
        b, g = divmod(c, 4)
        hs = slice(g * HD, (g + 1) * HD)
        in_maps.append({
            "xq": np.ascontiguousarray(query[b]),
            "xk": np.ascontiguousarray(key[b]),
            "xv": np.ascontiguousarray(value[b]),
            "wq": np.ascontiguousarray(Wq[:, hs]),
            "wk": np.ascontiguousarray(Wk[:, hs]),
            "wv": np.ascontiguousarray(Wv[:, hs]),
            "wo": np.ascontiguousarray(Wo[hs, :]),
            "bqkv": np.ascontiguousarray(
                np.stack([bq[hs], bk[hs], bv[hs]]).astype(np.float32)
            ),
        })

    res = run_bass_kernel_spmd(nc, in_maps, core_ids=list(range(8)), **_CACHE.get("run_kwargs", {}))
    _CACHE["last_result"] = res

    out = np.empty((B, S, D), dtype=np.float32)
    for b in range(B):
        acc = res.results[4 * b]["y"].astype(np.float32).copy()
        for g in range(1, 4):
            acc += res.results[4 * b + g]["y"]
        out[b] = acc + bo[None, :]
    return out
